# revision 5
# baseline (speedup 1.0000x reference)
"""v3: host pre-packs x and w into the exact SBUF layouts so every DMA is
fully contiguous per partition (16-32 KiB descriptors, no rearrange).

Layouts (per core, b_per_core=1024, P=128):
  xt [bt=8, p=128, mk=32, b=128]  xt[bt,p,m*2+k,b] = x[bt*128+b, m, k*128+p]
  wt [p=128, mk=32, o=256]        wt[p,m*2+k,o]    = w[m, k*128+p, o]

Device kernel per core: per bt (8): one contiguous 2 MiB load -> SBUF
[128p, 32mk, 128b]; per m: 2 accumulating matmuls (lhsT = x slice
[128i,128b], rhs = wt slice [128i,256o]) -> PSUM [128b, 256o]; DVE copy
-> staging [128b, 16m, 256o]; one contiguous 2 MiB store per bt.
"""

import numpy as np
from contextlib import ExitStack

import concourse.bass as bass
import concourse.tile as tile
import concourse.mybir as mybir
from concourse import bacc
from concourse.bass import ts
from concourse.bass_utils import run_bass_kernel_spmd

BATCH, M, D_IN, D_OUT = 8192, 16, 256, 256
N_CORES = 8
P = 128
KT = D_IN // P  # 2
MK = M * KT  # 32
F32 = mybir.dt.float32

_program_cache: dict = {}


def build_program(b_per_core: int, repeat: int = 1) -> bass.Bass:
    """repeat>1 re-runs the whole body (idempotent) — used only to measure
    true device time as the wall-clock slope over repeats."""
    key = (b_per_core, repeat)
    if key in _program_cache:
        return _program_cache[key]

    nc = bacc.Bacc("TRN2", target_bir_lowering=False, debug=False)

    n_btiles = b_per_core // P

    xt_ap = nc.dram_tensor(
        "xt", [n_btiles, P, MK, P], F32, kind="ExternalInput"
    ).ap()
    w_ap = nc.dram_tensor("w", [P, MK, D_OUT], F32, kind="ExternalInput").ap()
    o_ap = nc.dram_tensor(
        "out", [b_per_core, M * D_OUT], F32, kind="ExternalOutput"
    ).ap()

    with tile.TileContext(nc) as tc, ExitStack() as ctx:
        w_pool = ctx.enter_context(tc.tile_pool(name="w", bufs=1))
        x_pool = ctx.enter_context(tc.tile_pool(name="x", bufs=4))
        o_pool = ctx.enter_context(tc.tile_pool(name="o", bufs=4))
        pso_pool = ctx.enter_context(tc.tile_pool(name="pso", bufs=4, space="PSUM"))

        # Resident weights [128i, 32 mk, 256o]: one contiguous 4 MiB DMA.
        w_sb = w_pool.tile([P, MK, D_OUT], F32)
        nc.sync.dma_start(out=w_sb[:], in_=w_ap)

        for bt_r in range(n_btiles * repeat):
            bt = bt_r % n_btiles
            xts = x_pool.tile([P, MK, P], F32)
            nc.sync.dma_start(out=xts[:], in_=xt_ap[bt])
            ot = o_pool.tile([P, M * D_OUT], F32)

            # Two m's share one 2-bank PSUM tile -> one DVE copy per pair.
            for mp in range(M // 2):
                ps = pso_pool.tile([P, 2 * D_OUT], F32)
                for half in range(2):
                    m = 2 * mp + half
                    for k in range(KT):
                        nc.tensor.matmul(
                            ps[:, half * D_OUT : (half + 1) * D_OUT],
                            lhsT=xts[:, m * KT + k, :],
                            rhs=w_sb[:, m * KT + k, :],
                            start=(k == 0),
                            stop=(k == KT - 1),
                        )
                nc.vector.tensor_copy(
                    out=ot[:, mp * 2 * D_OUT : (mp + 1) * 2 * D_OUT], in_=ps[:]
                )

            nc.sync.dma_start(out=o_ap[ts(bt, P)], in_=ot[:])

    nc.compile()
    _program_cache[key] = nc
    return nc


def _host_transpose(x_shard: np.ndarray) -> np.ndarray:
    """[b, m, i] -> [bt, p, (m k), b] matching the SBUF tile layout."""
    b = x_shard.shape[0]
    return np.ascontiguousarray(
        x_shard.reshape(b // P, P, M, KT, P).transpose(0, 4, 2, 3, 1)
    ).reshape(b // P, P, MK, P)


def _host_pack_w(weights: np.ndarray) -> np.ndarray:
    """[m, i, o] -> [p, (m k), o]."""
    return np.ascontiguousarray(
        weights.reshape(M, KT, P, D_OUT).transpose(2, 0, 1, 3)
    ).reshape(P, MK, D_OUT)


def _run(x: np.ndarray, weights: np.ndarray, trace: bool = False):
    b_per_core = x.shape[0] // N_CORES
    nc = build_program(b_per_core)
    shards = np.split(x, N_CORES, axis=0)
    w = _host_pack_w(np.asarray(weights, dtype=np.float32))
    in_maps = [{"xt": _host_transpose(s), "w": w} for s in shards]
    res = run_bass_kernel_spmd(nc, in_maps, list(range(N_CORES)), trace=trace)
    out = np.concatenate(
        [r["out"].reshape(b_per_core, M, D_OUT) for r in res.results], axis=0
    )
    return out, res


def kernel(x: np.ndarray, weights: np.ndarray) -> np.ndarray:
    out, _ = _run(np.asarray(x), np.asarray(weights), trace=False)
    return out


# revision 7
# speedup vs baseline: 3.0184x; 3.0184x over previous
"""Grouped batched matmul out[b,m,o] = sum_i x[b,m,i] * w[m,i,o] on 8 TRN2
NeuronCores, data-parallel over batch (1024 rows/core), w replicated.

Design (per core):
- Host pre-packs x into xt[bt=8, p=128, (m k)=32, b=128] and w into
  wt[p=128, (m k)=32, o=256] — the exact SBUF layouts — so every DMA is a
  single fully-contiguous transfer (16-32 KiB per partition, clean 16 KiB
  descriptors).
- Inputs are declared float32r (same bits as fp32): fp32 matmuls run at 4
  cycles/row on the PE (two half-speed passes, ~109us/iter — co-bottleneck
  with DMA), fp32r runs at 1 cycle/row (~27us) and leaves the kernel purely
  HBM-bound. Measured rel err vs fp64 oracle: 1.5e-04 (gate 2e-2).
- Per bt: one 2 MiB x load; per m-pair: 4 accumulating matmuls (lhsT =
  x slice [128i,128b] stationary, rhs = w slice [128i,256o] moving) into a
  one-bank PSUM tile [128b,512o]; one DVE copy PSUM->SBUF staging; one
  contiguous 2 MiB store per bt.
- Steady state measured ~110us/iter/core = 8x(2 MiB load + 2 MiB store) at
  ~305 GB/s/core — at the practical HBM roofline (nominal 358 GB/s/core,
  shared 716 GB/s per 2-NC stack; all 8 cores concurrent).
"""

import numpy as np
from contextlib import ExitStack

import concourse.bass as bass
import concourse.tile as tile
import concourse.mybir as mybir
from concourse import bacc
from concourse.bass import ts
from concourse.bass_utils import run_bass_kernel_spmd

BATCH, M, D_IN, D_OUT = 8192, 16, 256, 256
N_CORES = 8
P = 128
KT = D_IN // P  # 2
MK = M * KT  # 32
F32 = mybir.dt.float32
F32R = mybir.dt.float32r

_program_cache: dict = {}


def build_program(b_per_core: int, repeat: int = 1) -> bass.Bass:
    """repeat>1 re-runs the whole body (idempotent) — used only to measure
    true device time as the wall-clock slope over repeats."""
    key = (b_per_core, repeat)
    if key in _program_cache:
        return _program_cache[key]

    nc = bacc.Bacc("TRN2", target_bir_lowering=False, debug=False)

    n_btiles = b_per_core // P

    xt_ap = nc.dram_tensor(
        "xt", [n_btiles, P, MK, P], F32R, kind="ExternalInput"
    ).ap()
    w_ap = nc.dram_tensor("w", [P, MK, D_OUT], F32R, kind="ExternalInput").ap()
    o_ap = nc.dram_tensor(
        "out", [b_per_core, M * D_OUT], F32, kind="ExternalOutput"
    ).ap()

    with tile.TileContext(nc) as tc, ExitStack() as ctx:
        w_pool = ctx.enter_context(tc.tile_pool(name="w", bufs=1))
        x_pool = ctx.enter_context(tc.tile_pool(name="x", bufs=4))
        o_pool = ctx.enter_context(tc.tile_pool(name="o", bufs=4))
        pso_pool = ctx.enter_context(tc.tile_pool(name="pso", bufs=4, space="PSUM"))

        # Resident weights [128i, 32 mk, 256o]: one contiguous 4 MiB DMA.
        w_sb = w_pool.tile([P, MK, D_OUT], F32R)
        nc.sync.dma_start(out=w_sb[:], in_=w_ap)

        for bt_r in range(n_btiles * repeat):
            bt = bt_r % n_btiles
            xts = x_pool.tile([P, MK, P], F32R)
            nc.sync.dma_start(out=xts[:], in_=xt_ap[bt])
            ot = o_pool.tile([P, M * D_OUT], F32)

            # Two m's share one 1-bank PSUM tile -> one DVE copy per pair.
            for mp in range(M // 2):
                ps = pso_pool.tile([P, 2 * D_OUT], F32)
                for half in range(2):
                    m = 2 * mp + half
                    for k in range(KT):
                        nc.tensor.matmul(
                            ps[:, half * D_OUT : (half + 1) * D_OUT],
                            lhsT=xts[:, m * KT + k, :],
                            rhs=w_sb[:, m * KT + k, :],
                            start=(k == 0),
                            stop=(k == KT - 1),
                        )
                nc.vector.tensor_copy(
                    out=ot[:, mp * 2 * D_OUT : (mp + 1) * 2 * D_OUT], in_=ps[:]
                )

            # Stores go through the ACT HWDGE ring (loads use SP's) so the
            # two descriptor streams don't share one FIFO.
            nc.scalar.dma_start(out=o_ap[ts(bt, P)], in_=ot[:])

    nc.compile()
    _program_cache[key] = nc
    return nc


def _host_transpose(x_shard: np.ndarray) -> np.ndarray:
    """[b, m, i] -> [bt, p, (m k), b] matching the SBUF tile layout."""
    b = x_shard.shape[0]
    return np.ascontiguousarray(
        x_shard.reshape(b // P, P, M, KT, P).transpose(0, 4, 2, 3, 1)
    ).reshape(b // P, P, MK, P)


def _host_pack_w(weights: np.ndarray) -> np.ndarray:
    """[m, i, o] -> [p, (m k), o]."""
    return np.ascontiguousarray(
        weights.reshape(M, KT, P, D_OUT).transpose(2, 0, 1, 3)
    ).reshape(P, MK, D_OUT)


def _run(x: np.ndarray, weights: np.ndarray, trace: bool = False):
    x = np.ascontiguousarray(np.asarray(x, dtype=np.float32))
    b_per_core = x.shape[0] // N_CORES
    nc = build_program(b_per_core)
    shards = np.split(x, N_CORES, axis=0)
    w = _host_pack_w(np.asarray(weights, dtype=np.float32))
    in_maps = [{"xt": _host_transpose(s), "w": w} for s in shards]
    res = run_bass_kernel_spmd(nc, in_maps, list(range(N_CORES)), trace=trace)
    out = np.concatenate(
        [r["out"].reshape(b_per_core, M, D_OUT) for r in res.results], axis=0
    )
    return out, res


def kernel(x: np.ndarray, weights: np.ndarray) -> np.ndarray:
    out, _ = _run(np.asarray(x), np.asarray(weights), trace=False)
    return out
